# revision 13
# baseline (speedup 1.0000x reference)
"""BLT local encoder (2-layer transformer, patch-equality block-diagonal attention)
on 8 Trainium2 NeuronCores.

Strategy: the attention mask is patch-equality over *sorted* patch_ids, i.e.
block-diagonal over contiguous runs. Each of the 4 sequences is split at a
patch boundary near S/2 into 2 fully independent shards -> 8 shards, one per
core, zero cross-core communication. Each shard (<=1152 tokens, padded) runs
the full encoder with the residual stream kept feature-major (transposed), so
every linear uses weight tiles as lhsT directly. Matmuls run in float32r
(full-rate fp32 PE mode). Attention is computed per 128-token tile against a
+-1-tile key window (patch runs are ~4-16 tokens, << 128).

SBUF static budget (per partition): hT 36K + b36(bufs=2) 72K + mid12(bufs=2)
24K + consts 8K + weight stream 12K + LN tmp 8K + attn small ~32K ~= 200K.
"""

import numpy as np

import concourse.bass as bass
import concourse.tile as tile
from concourse import bacc, bass_utils, mybir

F32 = mybir.dt.float32
F32R = mybir.dt.float32r
AF = mybir.ActivationFunctionType
OP = mybir.AluOpType

B, S, D, H, F, L = 4, 2048, 1024, 16, 4096, 2
DH = D // H  # 64
DC = D // 128  # 8
FC = F // 128  # 32
EPS = 1e-5
SCALE = 1.0 / np.sqrt(DH)

P = 128
NT = 9           # token tiles per shard
PT = NT * P      # 1152
TC = 384         # token chunk
NCH = 3
VP = 384
VC = 3
NCORES = 8


def _build():
    nc = bacc.Bacc("TRN2", target_bir_lowering=False, debug=False,
                   num_devices=NCORES)

    def din(name, shape, dt=F32):
        return nc.dram_tensor(name, shape, dt, kind="ExternalInput").ap()

    onehotT = din("onehotT", [P, VC * PT], F32R)
    tokemb = din("tokemb", [P, VC * D], F32R)
    baseT = din("baseT", [P, DC * PT], F32)
    masks_d = din("masks", [P, NT * 384], F32)
    ln0g = din("ln0g", [D]); ln0b = din("ln0b", [D])
    wq, wk, wv, wo, w1, w2 = [], [], [], [], [], []
    bq, bk, bv, bo, b1, b2, g1, n1, g2, n2 = [], [], [], [], [], [], [], [], [], []
    for l in range(L):
        wq.append(din(f"wq{l}", [D, D], F32R))
        wk.append(din(f"wk{l}", [D, D], F32R))
        wv.append(din(f"wv{l}", [D, D], F32R))
        wo.append(din(f"wo{l}", [D, D], F32R))
        w1.append(din(f"w1{l}", [D, F], F32R))
        w2.append(din(f"w2{l}", [F, D], F32R))
        bq.append(din(f"bq{l}", [D])); bk.append(din(f"bk{l}", [D]))
        bv.append(din(f"bv{l}", [D])); bo.append(din(f"bo{l}", [D]))
        b1.append(din(f"b1{l}", [F])); b2.append(din(f"b2{l}", [D]))
        g1.append(din(f"g1{l}", [D])); n1.append(din(f"n1{l}", [D]))
        g2.append(din(f"g2{l}", [D])); n2.append(din(f"n2{l}", [D]))
    houtT = nc.dram_tensor("houtT", [P, DC * PT], F32, kind="ExternalOutput").ap()

    with tile.TileContext(nc) as tc:
        with (
            tc.tile_pool(name="pers", bufs=1) as pers,
            tc.tile_pool(name="big", bufs=2) as big,
            tc.tile_pool(name="mid", bufs=2) as mid,
            tc.tile_pool(name="wp", bufs=3) as wp,
            tc.tile_pool(name="lnp", bufs=2) as lnp,
            tc.tile_pool(name="ap_", bufs=1) as ap_,
            tc.tile_pool(name="nrmp", bufs=2) as nrmp,
            tc.tile_pool(name="small", bufs=3) as small,
            tc.tile_pool(name="pp", bufs=8, space="PSUM") as pp,
        ):
            # ---------- constants (packed) ----------
            # cpack cols: 0 ones | 1 eps(row0) | 2:10 ln0g | 10:18 ln0b
            #   | per layer l at 18+96*l: bq 0:8 bk 8:16 bo 16:24 b2 24:32
            #     g1 32:40 n1 40:48 g2 48:56 n2 56:64 b1 64:96
            cpack = pers.tile([P, 224], F32, tag="cpack")
            nc.vector.memset(cpack[:, 0:1], 1.0)
            nc.vector.memset(cpack[0:1, 1:2], EPS)
            nc.sync.dma_start(out=cpack[:, 2:10], in_=ln0g.rearrange("(c p) -> p c", p=P))
            nc.sync.dma_start(out=cpack[:, 10:18], in_=ln0b.rearrange("(c p) -> p c", p=P))
            bcol = []
            for l in range(L):
                b0 = 18 + 96 * l
                for i, v in enumerate((bq[l], bk[l], bo[l], b2[l],
                                       g1[l], n1[l], g2[l], n2[l])):
                    nc.sync.dma_start(
                        out=cpack[:, b0 + 8 * i:b0 + 8 * i + 8],
                        in_=v.rearrange("(c p) -> p c", p=P))
                nc.sync.dma_start(out=cpack[:, b0 + 64:b0 + 96],
                                  in_=b1[l].rearrange("(c p) -> p c", p=P))
                bcol.append(cpack[:, b0:b0 + 96])
            eps_t = cpack[0:1, 1:2]
            ones_col = pers.tile([P, 1], F32R, tag="ones_col")
            nc.vector.tensor_copy(ones_col, cpack[:, 0:1])

            hT = pers.tile([P, DC * PT], F32, tag="hT")

            def ln_chunk(ci, gcol, bcol_, out_tile, out_stride):
                """LayerNorm over features (partitions) for token chunk ci."""
                t0 = ci * TC
                ps1 = pp.tile([1, TC], F32, tag="mm", name=f"lns1_{ci}")
                ps2 = pp.tile([1, TC], F32, tag="mm", name=f"lns2_{ci}")
                for dc in range(DC):
                    tmp = lnp.tile([P, TC], F32R, tag="lnt", name=f"lnt{dc}")
                    nc.vector.tensor_copy(tmp, hT[:, dc * PT + t0:dc * PT + t0 + TC])
                    nc.tensor.matmul(ps1, lhsT=ones_col, rhs=tmp,
                                     start=(dc == 0), stop=(dc == DC - 1))
                    sq = lnp.tile([P, TC], F32R, tag="lnt", name=f"lnsq{dc}")
                    nc.vector.tensor_mul(sq, tmp, tmp)
                    nc.tensor.matmul(ps2, lhsT=ones_col, rhs=sq,
                                     start=(dc == 0), stop=(dc == DC - 1))
                mean = small.tile([1, TC], F32, tag="sm", name="mean")
                nc.vector.tensor_scalar_mul(mean, ps1, 1.0 / D)
                var = small.tile([1, TC], F32, tag="sm", name="var")
                nc.vector.tensor_mul(var, mean, mean)
                nc.vector.scalar_tensor_tensor(var, ps2, 1.0 / D, var,
                                               op0=OP.mult, op1=OP.subtract)
                rstd = small.tile([1, TC], F32, tag="sm", name="rstd")
                nc.scalar.activation(rstd, var, AF.Sqrt, bias=eps_t)
                nc.vector.reciprocal(rstd, rstd)
                mr = small.tile([1, TC], F32, tag="sm", name="mr")
                nc.vector.tensor_mul(mr, mean, rstd)
                RM = ap_.tile([P, 2 * TC], F32, tag="lnRM")
                nc.gpsimd.partition_broadcast(RM[:, 0:TC], rstd[0:1, :])
                nc.gpsimd.partition_broadcast(RM[:, TC:2 * TC], mr[0:1, :])
                o0 = t0 if out_stride == PT else 0
                for dc in range(DC):
                    hsl = hT[:, dc * PT + t0:dc * PT + t0 + TC]
                    d1 = lnp.tile([P, TC], F32, tag="lnt", name=f"lnd{dc}")
                    nc.vector.tensor_mul(d1, hsl, RM[:, 0:TC])
                    d2 = lnp.tile([P, TC], F32, tag="lnt", name=f"lnd2_{dc}")
                    nc.vector.tensor_sub(d2, d1, RM[:, TC:2 * TC])
                    osl = out_tile[:, dc * out_stride + o0:dc * out_stride + o0 + TC]
                    nc.vector.tensor_scalar(
                        osl, d2, gcol[:, dc:dc + 1], bcol_[:, dc:dc + 1],
                        op0=OP.mult, op1=OP.add)

            # ---------- preamble: embeddings + LN0 ----------
            oht = big.tile([P, VC * PT], F32R, tag="b36", name="oht")
            nc.sync.dma_start(out=oht, in_=onehotT)
            tet = big.tile([P, VC * D], F32R, tag="b36", name="tet")
            nc.sync.dma_start(out=tet, in_=tokemb)
            for dc in range(DC):
                nc.sync.dma_start(out=hT[:, dc * PT:(dc + 1) * PT],
                                  in_=baseT[:, dc * PT:(dc + 1) * PT])
            for ci in range(NCH):
                t0 = ci * TC
                for dc in range(DC):
                    pse = pp.tile([P, TC], F32, tag="mm", name=f"pse{dc}")
                    for vc in range(VC):
                        nc.tensor.matmul(
                            pse,
                            lhsT=tet[:, vc * D + dc * 128:vc * D + dc * 128 + 128],
                            rhs=oht[:, vc * PT + t0:vc * PT + t0 + TC],
                            start=(vc == 0), stop=(vc == VC - 1))
                    hsl = hT[:, dc * PT + t0:dc * PT + t0 + TC]
                    nc.vector.tensor_add(hsl, pse, hsl)
            for ci in range(NCH):
                ln_chunk(ci, cpack[:, 2:10], cpack[:, 10:18], hT, PT)

            # ---------- layers ----------
            for l in range(L):
                KT = big.tile([P, DC * PT], F32R, tag="b36", name=f"KT{l}")
                Vsb = big.tile([P, NT * H, DH], F32R, tag="b36", name=f"Vsb{l}")
                bvb = ap_.tile([P, D], F32, tag="bvb")
                nc.sync.dma_start(
                    out=bvb,
                    in_=bass.AP(tensor=bv[l].tensor, offset=bv[l].offset,
                                ap=[[0, P]] + list(bv[l].ap)))

                # ---- K and V (full shard) ----
                for ci in range(NCH):
                    t0 = ci * TC
                    xh = mid.tile([P, DC * TC], F32R, tag="m12", name=f"xh{ci}")
                    ln_chunk(ci, bcol[l][:, 32:40], bcol[l][:, 40:48], xh, TC)
                    pss = [pp.tile([P, TC], F32, tag="mm", name=f"psk{i}")
                           for i in range(DC)]
                    for dc in range(DC):
                        wb = wp.tile([P, D], F32R, tag="w", name=f"wkb{dc}")
                        nc.sync.dma_start(out=wb, in_=wk[l][dc * 128:(dc + 1) * 128, :])
                        for oc in range(DC):
                            nc.tensor.matmul(
                                pss[oc], lhsT=wb[:, oc * 128:oc * 128 + 128],
                                rhs=xh[:, dc * TC:(dc + 1) * TC],
                                start=(dc == 0), stop=(dc == DC - 1))
                    for oc in range(DC):
                        nc.vector.tensor_scalar_add(
                            KT[:, oc * PT + t0:oc * PT + t0 + TC], pss[oc],
                            bcol[l][:, 8 + oc:8 + oc + 1])
                    psv = [pp.tile([P, 512], F32, tag="mm", name=f"psv{i}")
                           for i in range(6)]
                    for dc in range(DC):
                        wb = wp.tile([P, D], F32R, tag="w", name=f"wvb{dc}")
                        nc.sync.dma_start(out=wb, in_=wv[l][dc * 128:(dc + 1) * 128, :])
                        for tt in range(3):
                            for nh in range(2):
                                nc.tensor.matmul(
                                    psv[tt * 2 + nh],
                                    lhsT=xh[:, dc * TC + tt * 128:dc * TC + tt * 128 + 128],
                                    rhs=wb[:, nh * 512:(nh + 1) * 512],
                                    start=(dc == 0), stop=(dc == DC - 1))
                    for tt in range(3):
                        g = 3 * ci + tt
                        for nh in range(2):
                            pv = psv[tt * 2 + nh][:, :].rearrange(
                                "p (h x) -> p h x", h=8)
                            bvv = bvb[:, nh * 512:(nh + 1) * 512].rearrange(
                                "p (h x) -> p h x", h=8)
                            ov = Vsb[:, g * H + nh * 8:g * H + nh * 8 + 8, :]
                            nc.vector.tensor_add(ov, pv, bvv)

                # ---- attention (per chunk: recompute LN+Q, then attend) ----
                for c in range(NCH):
                    t0 = c * TC
                    xh = mid.tile([P, DC * TC], F32R, tag="m12", name=f"axh{c}")
                    ln_chunk(c, bcol[l][:, 32:40], bcol[l][:, 40:48], xh, TC)
                    QTc = mid.tile([P, DC * TC], F32R, tag="m12", name=f"qtc{c}")
                    psq = [pp.tile([P, TC], F32, tag="mm", name=f"psq{i}")
                           for i in range(DC)]
                    for dc in range(DC):
                        wb = wp.tile([P, D], F32R, tag="w", name=f"wqb{dc}")
                        nc.sync.dma_start(out=wb, in_=wq[l][dc * 128:(dc + 1) * 128, :])
                        for oc in range(DC):
                            nc.tensor.matmul(
                                psq[oc], lhsT=wb[:, oc * 128:oc * 128 + 128],
                                rhs=xh[:, dc * TC:(dc + 1) * TC],
                                start=(dc == 0), stop=(dc == DC - 1))
                    for oc in range(DC):
                        nc.vector.tensor_scalar_add(
                            QTc[:, oc * TC:(oc + 1) * TC], psq[oc],
                            bcol[l][:, oc:oc + 1])

                    ctxc = mid.tile([P, DC * TC], F32R, tag="m12", name=f"ctx{c}")
                    kts = [j for j in range(3 * c - 1, 3 * c + 4) if 0 <= j < NT]
                    mk = ap_.tile([P, 5 * 384], F32, tag="mk")
                    for jj, j in enumerate(kts):
                        nc.sync.dma_start(out=mk[:, jj * 384:(jj + 1) * 384],
                                          in_=masks_d[:, j * 384:(j + 1) * 384])
                    for h in range(H):
                        dch, po = h // 2, (h % 2) * 64
                        est = ap_.tile([P, 5 * 384], F32R, tag="est")
                        for jj, j in enumerate(kts):
                            lo = max(3 * c, j - 1)
                            hi = min(3 * c + 2, j + 1)
                            nq = (hi - lo + 1) * 128
                            w0t = min(max(j - 1, 0), NT - 3)
                            pst = pp.tile([P, 384], F32, tag="mm", name=f"pst{jj}")
                            nc.tensor.matmul(
                                pst[:, 0:nq],
                                lhsT=KT[po:po + 64, dch * PT + j * 128:dch * PT + j * 128 + 128],
                                rhs=QTc[po:po + 64, dch * TC + (lo - 3 * c) * 128:dch * TC + (lo - 3 * c) * 128 + nq],
                                start=True, stop=True)
                            esl = est[:, jj * 384:jj * 384 + nq]
                            nc.scalar.activation(esl, pst[:, 0:nq], AF.Exp,
                                                 scale=float(SCALE))
                            mo = jj * 384 + (lo - w0t) * 128
                            nc.vector.tensor_mul(esl, esl, mk[:, mo:mo + nq])
                        psc = pp.tile([64, 384], F32, tag="mm", name=f"psc{h}")
                        psd = pp.tile([1, 384], F32, tag="mm", name=f"psd{h}")
                        for qi in range(3):
                            qt = 3 * c + qi
                            js = [j for j in (qt - 1, qt, qt + 1) if 0 <= j < NT]
                            for kk, j in enumerate(js):
                                jj = kts.index(j)
                                lo_j = max(3 * c, j - 1)
                                qoff = (qt - lo_j) * 128
                                rsl = est[:, jj * 384 + qoff:jj * 384 + qoff + 128]
                                nc.tensor.matmul(
                                    psc[:, qi * 128:(qi + 1) * 128],
                                    lhsT=Vsb[:, j * H + h, :], rhs=rsl,
                                    start=(kk == 0), stop=(kk == len(js) - 1))
                                nc.tensor.matmul(
                                    psd[:, qi * 128:(qi + 1) * 128],
                                    lhsT=ones_col, rhs=rsl,
                                    start=(kk == 0), stop=(kk == len(js) - 1))
                        # normalize: nrm cols 0:384 bcast area, 384:768 row0 rcp
                        nrm = nrmp.tile([P, 2 * 384], F32, tag="nrm")
                        nc.vector.reciprocal(nrm[0:1, 384:768], psd[:, :])
                        nc.gpsimd.partition_broadcast(nrm[0:64, 0:384],
                                                      nrm[0:1, 384:768])
                        nc.vector.tensor_mul(
                            ctxc[po:po + 64, dch * TC:dch * TC + TC],
                            psc[:, :], nrm[0:64, 0:384])
                    # O-projection + residual
                    pso = [pp.tile([P, TC], F32, tag="mm", name=f"pso{i}")
                           for i in range(DC)]
                    for di in range(DC):
                        wb = wp.tile([P, D], F32R, tag="w", name=f"wob{di}")
                        nc.sync.dma_start(out=wb, in_=wo[l][di * 128:(di + 1) * 128, :])
                        for do_ in range(DC):
                            nc.tensor.matmul(
                                pso[do_], lhsT=wb[:, do_ * 128:do_ * 128 + 128],
                                rhs=ctxc[:, di * TC:(di + 1) * TC],
                                start=(di == 0), stop=(di == DC - 1))
                    for do_ in range(DC):
                        hsl = hT[:, do_ * PT + t0:do_ * PT + t0 + TC]
                        nc.vector.scalar_tensor_tensor(
                            hsl, pso[do_], bcol[l][:, 16 + do_:16 + do_ + 1], hsl,
                            op0=OP.add, op1=OP.add)

                # ---- FFN ----
                for ci in range(NCH):
                    t0 = ci * TC
                    xh = mid.tile([P, DC * TC], F32R, tag="m12", name=f"fxh{ci}")
                    ln_chunk(ci, bcol[l][:, 48:56], bcol[l][:, 56:64], xh, TC)
                    uTa = big.tile([P, 16 * TC], F32R, tag="b36", name=f"uTa{ci}")
                    uTb = big.tile([P, 16 * TC], F32R, tag="b36", name=f"uTb{ci}")

                    def usl(fc):
                        t = uTa if fc < 16 else uTb
                        k = fc % 16
                        return t[:, k * TC:(k + 1) * TC]

                    for fg in range(4):
                        psf = [pp.tile([P, TC], F32, tag="mm", name=f"psf{i}")
                               for i in range(8)]
                        for dc in range(DC):
                            wb = wp.tile([P, D], F32R, tag="w", name=f"w1b{dc}")
                            nc.sync.dma_start(
                                out=wb,
                                in_=w1[l][dc * 128:(dc + 1) * 128, fg * 1024:(fg + 1) * 1024])
                            for fcl in range(8):
                                nc.tensor.matmul(
                                    psf[fcl], lhsT=wb[:, fcl * 128:fcl * 128 + 128],
                                    rhs=xh[:, dc * TC:(dc + 1) * TC],
                                    start=(dc == 0), stop=(dc == DC - 1))
                        for fcl in range(8):
                            fc = fg * 8 + fcl
                            nc.scalar.activation(
                                usl(fc), psf[fcl], AF.Gelu,
                                bias=bcol[l][:, 64 + fc:64 + fc + 1])
                    psh = [pp.tile([P, TC], F32, tag="mm", name=f"psh{i}")
                           for i in range(DC)]
                    for fc in range(FC):
                        wb = wp.tile([P, D], F32R, tag="w", name=f"w2b{fc}")
                        nc.sync.dma_start(out=wb, in_=w2[l][fc * 128:(fc + 1) * 128, :])
                        for do_ in range(DC):
                            nc.tensor.matmul(
                                psh[do_], lhsT=wb[:, do_ * 128:do_ * 128 + 128],
                                rhs=usl(fc),
                                start=(fc == 0), stop=(fc == FC - 1))
                    for do_ in range(DC):
                        hsl = hT[:, do_ * PT + t0:do_ * PT + t0 + TC]
                        nc.vector.scalar_tensor_tensor(
                            hsl, psh[do_], bcol[l][:, 24 + do_:24 + do_ + 1], hsl,
                            op0=OP.add, op1=OP.add)

            nc.sync.dma_start(out=houtT, in_=hT[:])

    nc.compile()
    return nc


_NC_CACHE = {}


def _get_nc():
    if "nc" not in _NC_CACHE:
        _NC_CACHE["nc"] = _build()
    return _NC_CACHE["nc"]


def _prep_core(inputs, b, start, n):
    """Per-core in_map entries that depend on the shard."""
    ids = np.asarray(inputs["input_ids"][b, start:start + n])
    pid = np.asarray(inputs["patch_ids"][b, start:start + n]).astype(np.int64)
    pos_emb = np.asarray(inputs["pos_emb"], np.float32)
    hashes = np.asarray(inputs["hash_embeddings"], np.float32)

    oh = np.zeros((VP, PT), np.float32)
    oh[ids, np.arange(n)] = 1.0
    onehotT = np.ascontiguousarray(
        oh.reshape(VC, P, PT).transpose(1, 0, 2).reshape(P, VC * PT))

    base = np.zeros((PT, D), np.float32)
    base[:n] = pos_emb[start:start + n] + hashes[b, start:start + n]
    baseT = np.ascontiguousarray(
        base.reshape(PT, DC, P).transpose(2, 1, 0).reshape(P, DC * PT))

    pidp = np.empty(PT, np.int64)
    pidp[:n] = pid
    pidp[n:] = -np.arange(1, PT - n + 1)

    m = np.zeros((NT, P, 384), np.float32)
    for j in range(NT):
        w0 = np.clip(j - 1, 0, NT - 3) * P
        kk = pidp[j * P:(j + 1) * P]
        qq = pidp[w0:w0 + 384]
        m[j] = (kk[:, None] == qq[None, :]).astype(np.float32)
    masks = np.ascontiguousarray(m.transpose(1, 0, 2).reshape(P, NT * 384))
    return {"onehotT": onehotT, "baseT": baseT, "masks": masks}


def kernel(**inputs):
    pid_all = np.asarray(inputs["patch_ids"])
    tok = np.asarray(inputs["tok_emb"], np.float32)
    tokp = np.zeros((VP, D), np.float32)
    tokp[:tok.shape[0]] = tok
    tokemb = np.ascontiguousarray(
        tokp.reshape(VC, P, D).transpose(1, 0, 2).reshape(P, VC * D))

    shared = {"tokemb": tokemb,
              "ln0g": np.ascontiguousarray(np.asarray(inputs["ln0_g"], np.float32)),
              "ln0b": np.ascontiguousarray(np.asarray(inputs["ln0_b"], np.float32))}
    for l in range(L):
        for nm, key in (("wq", "Wq"), ("wk", "Wk"), ("wv", "Wv"), ("wo", "Wo"),
                        ("w1", "W1"), ("w2", "W2"), ("bq", "bq"), ("bk", "bk"),
                        ("bv", "bv"), ("bo", "bo"), ("b1", "b1"), ("b2", "b2"),
                        ("g1", "ln1_g"), ("n1", "ln1_b"), ("g2", "ln2_g"),
                        ("n2", "ln2_b")):
            shared[f"{nm}{l}"] = np.ascontiguousarray(
                np.asarray(inputs[key][l], np.float32))

    shards = []
    for b in range(B):
        pid = np.asarray(pid_all[b])
        bnd = np.nonzero(pid[1:] != pid[:-1])[0] + 1
        cand = bnd[(bnd >= S - PT) & (bnd <= PT)]
        if len(cand) == 0:
            raise RuntimeError("no patch boundary near S/2; cannot shard")
        s = int(cand[np.argmin(np.abs(cand - S // 2))])
        shards.append((b, 0, s))
        shards.append((b, s, S - s))

    in_maps = []
    for b, start, n in shards:
        m = dict(shared)
        m.update(_prep_core(inputs, b, start, n))
        in_maps.append(m)

    nc = _get_nc()
    res = bass_utils.run_bass_kernel_spmd(nc, in_maps, core_ids=list(range(NCORES)))

    out = np.zeros((B, S, D), np.float32)
    for i, (b, start, n) in enumerate(shards):
        ht = res.results[i]["houtT"]
        hfull = ht.reshape(P, DC, PT).transpose(2, 1, 0).reshape(PT, D)
        out[b, start:start + n] = hfull[:n]
    return out


if __name__ == "__main__":
    _get_nc()
    print("built ok")
